# revision 44
# baseline (speedup 1.0000x reference)
"""CAM (channel attention) module kernel for Trainium2, 8-core data-parallel.

Computes, per batch b (one batch per NeuronCore):
    q = x[b].reshape(C, N)                  # C=512, N=4096
    E = q @ q.T                             # [C, C], symmetric
    att = softmax(rowmax(E) - E, axis=-1)   # == exp(rowmin(E)-E)/rowsum
    out = gamma * (att @ q) + x[b]

v3 design notes (evolved from the v2 xbar-transpose pipeline):
  - fp32 matmul on trn2 is 2-pass (4 cyc/row) -> all matmuls bf16 (1 cyc/row);
    the +x add and gamma/s scaling stay fp32 exact via one DVE
    scalar_tensor_tensor per output chunk (gamma=0 -> out == x exactly).
  - No DMA crossbar transposes at all: during the load phase the DMA engines
    are saturated by the 8MB x load (358GB/s), so every transpose (qT k-tiles
    and attT) runs on the PE as bf16 is_transpose matmuls; LDWEIGHTS hides
    under the moving passes.
  - Loads ride SP+ACT HWDGE rings in 4 column groups of [10,10,8,4] k-tiles:
    the small last group keeps the energy tail (which can only start once the
    last chunk is resident) short. Casts fp32->bf16 and PSUM->SBUF transpose
    gathers are split across DVE/ACT per chunk.
  - Energy uses symmetry: only column blocks j >= i accumulate (10/16 of the
    FLOPs); the j < i blocks are mirrored from finished row blocks with fp32
    PE transposes. The last group's energy runs i-outer so E[0] completes
    (and softmax(0) starts) before E[3]'s tail.
  - Per slab i: rowmin (DVE), exp(mn-E) with fused row-sum (ACT, bf16 att),
    attT = 4 bf16 PE transposes into the PSUM bank E[i] just freed by exp,
    gathered to SBUF by ACT. rg = gamma/rowsum stays per-partition and rides
    the final STT, so att is left unnormalized.
  - Out slab i: 4 chunks of [128,1024]: 8 accumulating matmuls (lhsT = attT
    blocks, rhs = q_bf) -> STT (psum*rg + x, DVE) -> store on the SP ring.
    Slab i+1's mirror/rowmin/exp/attT are hand-interleaved under slab i's
    matmuls so the PE never idles (and keeps its 2.4GHz p-state).
"""

import sys

import numpy as np

for _p in ("/opt/trn_rl_repo",):
    if _p not in sys.path:
        sys.path.insert(0, _p)

B, C, H, W = 8, 512, 64, 64
N = H * W  # 4096
P = 128
CT = C // P  # 4 channel tiles
KT = N // P  # 32 spatial k-tiles
FD = 512  # PSUM bank width (fp32)
KS = [10, 10, 6, 6]  # k-tiles per load group (small tail groups)
K0 = [0, 10, 20, 26]

_CACHE = {}


def _build_bass():
    import concourse.mybir as mybir
    import concourse.tile as tile
    from concourse import bacc
    from concourse.masks import make_identity

    fp32 = mybir.dt.float32
    bf16 = mybir.dt.bfloat16
    AX = mybir.AxisListType.X
    ALU = mybir.AluOpType
    ACT_EXP = mybir.ActivationFunctionType.Exp

    nc = bacc.Bacc(None, target_bir_lowering=False, debug=False)
    x_d = nc.dram_tensor("x", [C, N], fp32, kind="ExternalInput")
    g_d = nc.dram_tensor("gamma", [1], fp32, kind="ExternalInput")
    o_d = nc.dram_tensor("out", [C, N], fp32, kind="ExternalOutput")

    with tile.TileContext(nc) as tc:
        with (
            tc.tile_pool(name="persist", bufs=1) as persist,
            tc.tile_pool(name="stats", bufs=4) as stats,
            tc.tile_pool(name="outp", bufs=4) as outp,
            tc.tile_pool(name="epsum", bufs=4, space="PSUM") as epsum,
            tc.tile_pool(name="opsum", bufs=2, space="PSUM") as opsum,
        ):
            gam = persist.tile([P, 1], fp32)
            ident = persist.tile([P, P], bf16)
            ident32 = persist.tile([P, P], fp32)
            q = persist.tile([P, CT, N], fp32)
            q_bf = persist.tile([P, CT, N], bf16)
            # k-major qT: qT[p, k, c, v] = q[c*128+v, k*128+p]; energy rhs for
            # k is the contiguous [128, (4-i)*128] slab qT[:, k, i:, :]
            qT = persist.tile([P, KT, CT, P], bf16)
            att = persist.tile([P, CT, C], bf16)
            attT = persist.tile([P, CT, CT, P], bf16)

            nc.gpsimd.dma_start(out=gam, in_=g_d[:].to_broadcast((P, 1)))
            make_identity(nc, ident)
            make_identity(nc, ident32)

            def gsl(g, w=1):
                return slice(K0[g] * P * w, (K0[g] + KS[g]) * P * w)

            def load(g, c):
                ring = nc.sync if c % 2 == 0 else nc.scalar
                ring.dma_start(
                    out=q[:, c, gsl(g)], in_=x_d[c * P : (c + 1) * P, gsl(g)]
                )

            def load_halves(g, c):
                # split one chunk across both rings -> chunks complete
                # serially ~2x sooner each, for a faster pipeline ramp-in
                w = KS[g] * P
                lo, mid, hi = K0[g] * P, K0[g] * P + w // 2, (K0[g] + KS[g]) * P
                nc.sync.dma_start(
                    out=q[:, c, lo:mid], in_=x_d[c * P : (c + 1) * P, lo:mid]
                )
                nc.scalar.dma_start(
                    out=q[:, c, mid:hi], in_=x_d[c * P : (c + 1) * P, mid:hi]
                )

            def cast(g, c):
                # the last chunks' casts ride ACT: DVE's queue is congested
                # late in phase A and these casts gate the energy tail
                if c == 3 and g >= 2:
                    nc.scalar.copy(out=q_bf[:, c, gsl(g)], in_=q[:, c, gsl(g)])
                else:
                    nc.vector.tensor_copy(
                        out=q_bf[:, c, gsl(g)], in_=q[:, c, gsl(g)]
                    )

            def trans(g, c):
                # 1 PE transpose per k-tile of this chunk into a PSUM staging
                # tile (shared ring with the out-phase psum tiles)
                tp = opsum.tile([P, KS[g] * P], bf16, name="tp", tag="ops")
                for kk in range(KS[g]):
                    nc.tensor.transpose(
                        tp[:, kk * P : (kk + 1) * P],
                        q_bf[:, c, (K0[g] + kk) * P : (K0[g] + kk + 1) * P],
                        ident,
                    )
                return tp

            def gather(g, c, tp):
                # gathers for c in {1,3} ride ACT (its ring-issue slack);
                # {0,2} stay on DVE alongside the casts
                dst = qT[:, K0[g] : K0[g] + KS[g], c, :]
                src = tp[:, 0 : KS[g] * P].rearrange("p (k v) -> p k v", v=P)
                if c % 2 == 0:
                    nc.vector.tensor_copy(out=dst, in_=src)
                else:
                    nc.scalar.copy(out=dst, in_=src)

            Es = [
                epsum.tile([P, C], fp32, name=f"E{i}", tag=f"E{i}", bufs=1)
                for i in range(CT)
            ]

            def energy(ks, i_list=range(CT), stop=False):
                for k in ks:
                    for i in i_list:
                        nc.tensor.matmul(
                            Es[i][:, i * P :],
                            lhsT=qT[:, k, i, :],
                            rhs=qT[:, k, i:, :],
                            start=(k == 0),
                            stop=(stop and k == KT - 1),
                        )

            # ---- phase A ----
            # Emission is hand-interleaved so the PE never drains (idle gaps
            # reset the 2.4GHz p-state ramp): energy k-quads of group g-1 fill
            # the waits between group g's chunk transposes, whose casts/
            # gathers arrive chunk-by-chunk off the load rings.
            def prep_chunk(g, c):
                tp = trans(g, c)
                gather(g, c, tp)

            def block(ge, gt):
                # energy group ge interleaved with transposes of group gt;
                # trailing quads after T(gt,3) cover the last gather's latency
                # so the next block's energy never stalls the PE
                ks = list(range(K0[ge], K0[ge] + KS[ge]))
                n = len(ks)
                cuts = [n - 8, n - 6, n - 4, n - 2] if n >= 8 else [1, 2, 3, 4]
                cast(gt, 0)
                cast(gt, 1)
                lo = 0
                for c in range(CT):
                    energy(ks[lo : cuts[c]])
                    lo = cuts[c]
                    if c == 1:
                        cast(gt, 2)
                    if c == 2:
                        cast(gt, 3)
                    prep_chunk(gt, c)
                energy(ks[lo:])

            def lower_rows(g, c):
                # early-group energy as LOWER-triangle block rows: row c
                # spans blocks (c, 0..c), needing only chunks <= c -- energy
                # tracks the load stream chunk-by-chunk at full moving width.
                # The mirror transposes later ACCUMULATE the upper partials
                # on top, so E comes out identical. At the last lower k the
                # j<c columns stop (lmirror later overwrites them) while the
                # diagonal continues into the upper passes.
                for k in range(K0[g], K0[g] + KS[g]):
                    last = g == 1 and k == K0[1] + KS[1] - 1
                    if last and c > 0:
                        nc.tensor.matmul(
                            Es[c][:, 0 : c * P],
                            lhsT=qT[:, k, c, :],
                            rhs=qT[:, k, 0:c, :],
                            start=False,
                            stop=True,
                        )
                        nc.tensor.matmul(
                            Es[c][:, c * P : (c + 1) * P],
                            lhsT=qT[:, k, c, :],
                            rhs=qT[:, k, c : c + 1, :],
                            start=False,
                            stop=False,
                        )
                    else:
                        nc.tensor.matmul(
                            Es[c][:, 0 : (c + 1) * P],
                            lhsT=qT[:, k, c, :],
                            rhs=qT[:, k, 0 : c + 1, :],
                            start=(k == 0),
                            stop=False,
                        )

            # ---- transition + per-slab softmax/out, hand-interleaved ----
            mn = [stats.tile([P, 1], fp32, name=f"mn{i}", tag=f"mn{i}", bufs=1)
                  for i in range(CT)]
            s = [stats.tile([P, 1], fp32, name=f"s{i}", tag=f"s{i}", bufs=1)
                 for i in range(CT)]
            rg = [stats.tile([P, 1], fp32, name=f"rg{i}", tag=f"rg{i}", bufs=1)
                  for i in range(CT)]
            etmp = {}

            # E assembly: groups 0-1 accumulate LOWER-triangle block rows
            # (arrival-matched); at g1-end each lower off-diag block (j,i) is
            # transposed to SEED the upper region (i,j) (start=True), which
            # groups 2-3 then accumulate onto. The tail mirrors (i, j<i)
            # from the completed (j,i) blocks by overwrite, as usual.
            def etl_copy(j, i):
                # stage lower block (j,i), final after lower_rows(1, j)
                t = stats.tile([P, P], fp32, name=f"el{j}{i}", tag=f"el{j}{i}",
                               bufs=1)
                etmp[("l", j, i)] = t
                nc.scalar.copy(out=t, in_=Es[j][:, i * P : (i + 1) * P])

            def seed_upper(i, j):
                # (i,j) j>i := transpose(lower (j,i)) as a REGULAR fp32
                # matmul against the identity (etl^T @ I): unlike a transpose
                # op, this arms the PSUM accumulation group on real hardware
                # so the upper k-passes accumulate onto it correctly.
                nc.tensor.matmul(
                    Es[i][:, j * P : (j + 1) * P], lhsT=etmp[("l", j, i)],
                    rhs=ident32, start=True, stop=False,
                )

            def etu_copy(j, i):
                # stage FULL block (j,i) (j < i), final after Etail(j)
                t = stats.tile([P, P], fp32, name=f"eu{j}{i}", tag=f"eu{j}{i}",
                               bufs=1)
                etmp[("u", j, i)] = t
                nc.scalar.copy(out=t, in_=Es[j][:, i * P : (i + 1) * P])

            def mirror(i, j):
                # overwrite (i, j<i) with transpose of the full block (j,i)
                nc.tensor.matmul(
                    Es[i][:, j * P : (j + 1) * P], lhsT=etmp[("u", j, i)],
                    rhs=ident32, is_transpose=True, start=True, stop=True,
                )

            def rowmin(i):
                nc.vector.tensor_reduce(out=mn[i], in_=Es[i], axis=AX, op=ALU.min)

            def exp(i):
                nc.scalar.activation(
                    out=att[:, i, :], in_=Es[i], func=ACT_EXP, bias=mn[i],
                    scale=-1.0, accum_out=s[i],
                )

            def recip(i):
                nc.vector.reciprocal(out=rg[i], in_=s[i])
                nc.vector.tensor_mul(rg[i], rg[i], gam)

            def attT_T(i):
                # reuse E[i]'s PSUM bank (freed by exp(i)) as attT staging
                atp = epsum.tile([P, 2 * C], bf16, name=f"atp{i}", tag=f"E{i}",
                                 bufs=1)
                for j in range(CT):
                    nc.tensor.transpose(
                        atp[:, j * P : (j + 1) * P],
                        att[:, i, j * P : (j + 1) * P],
                        ident,
                    )
                return atp

            def attT_gather(i, atp):
                nc.scalar.copy(
                    out=attT[:, i, :, :],
                    in_=atp[:, 0:C].rearrange("p (j v) -> p j v", v=P),
                )

            def out_chunk(i, nh, split_tail=False):
                sl = slice(nh * 2 * FD, (nh + 1) * 2 * FD)
                ops = opsum.tile([P, 2 * FD], fp32, name="ops", tag="ops")
                for half in range(2):
                    hsl = slice((nh * 2 + half) * FD, (nh * 2 + half + 1) * FD)
                    for j in range(CT):
                        nc.tensor.matmul(
                            ops[:, half * FD : (half + 1) * FD],
                            lhsT=attT[:, i, j, :],
                            rhs=q_bf[:, j, hsl],
                            start=(j == 0),
                            stop=(j == CT - 1),
                        )
                ot = outp.tile([P, 2 * FD], fp32, name="ot", tag="ot")
                if not split_tail:
                    nc.vector.scalar_tensor_tensor(
                        out=ot, in0=ops, scalar=rg[i], in1=q[:, i, sl],
                        op0=ALU.mult, op1=ALU.add,
                    )
                    nc.sync.dma_start(out=o_d[i * P : (i + 1) * P, sl], in_=ot)
                    return
                # last chunk: halve the STT+store so the final drain is short
                for half in range(2):
                    hsl = slice((nh * 2 + half) * FD, (nh * 2 + half + 1) * FD)
                    nc.vector.scalar_tensor_tensor(
                        out=ot[:, half * FD : (half + 1) * FD],
                        in0=ops[:, half * FD : (half + 1) * FD],
                        scalar=rg[i], in1=q[:, i, hsl],
                        op0=ALU.mult, op1=ALU.add,
                    )
                    nc.sync.dma_start(
                        out=o_d[i * P : (i + 1) * P, hsl],
                        in_=ot[:, half * FD : (half + 1) * FD],
                    )

            for c in range(CT):
                load_halves(0, c)
            for c in range(CT):
                load(1, c)
            cast(0, 0)
            cast(0, 1)
            prep_chunk(0, 0)
            cast(0, 2)
            prep_chunk(0, 1)
            cast(0, 3)
            prep_chunk(0, 2)
            prep_chunk(0, 3)
            for c in range(CT):
                load(2, c)
            block(0, 1)
            for c in range(CT):
                load_halves(3, c)
            block(1, 2)
            block(2, 3)

            tail = range(K0[3], KT)  # 4 k-tiles per row block
            # E tails i-outer (row 0 completes first: its j>0 blocks were
            # seeded+accumulated in place); mirrors rebuild (i, j<i) from the
            # completed rows. rowmins early on DVE so the STT stream never
            # waits on softmax prep.
            energy(tail, i_list=[0], stop=True)      # PE: row 0 complete
            rowmin(0)                                # DVE
            etu_copy(0, 1)                           # ACT
            etu_copy(0, 2)
            etu_copy(0, 3)
            energy(tail, i_list=[1], stop=True)      # PE
            mirror(1, 0)                             # PE (from etu(0,1))
            exp(0)                                   # ACT (waits rowmin 0)
            recip(0)                                 # DVE
            rowmin(1)                                # DVE
            etu_copy(1, 2)                           # ACT
            etu_copy(1, 3)
            energy(tail, i_list=[2], stop=True)      # PE
            mirror(2, 0)                             # PE
            mirror(2, 1)
            energy(tail, i_list=[3], stop=True)      # PE
            etu_copy(2, 3)                           # ACT
            atp0 = attT_T(0)                         # PE (waits exp 0)
            attT_gather(0, atp0)                     # ACT
            mirror(3, 0)
            mirror(3, 1)
            mirror(3, 2)
            rowmin(2)                                # DVE
            rowmin(3)                                # DVE
            exp(1)                                   # ACT (waits rowmin 1)

            out_chunk(0, 0)
            out_chunk(0, 1)
            atp1 = attT_T(1)                         # PE
            attT_gather(1, atp1)                     # ACT
            recip(1)                                 # DVE
            exp(2)                                   # ACT
            out_chunk(0, 2)
            out_chunk(0, 3)
            out_chunk(1, 0)
            out_chunk(1, 1)
            atp2 = attT_T(2)                         # PE
            attT_gather(2, atp2)                     # ACT
            recip(2)                                 # DVE
            exp(3)                                   # ACT
            out_chunk(1, 2)
            out_chunk(1, 3)
            out_chunk(2, 0)
            out_chunk(2, 1)
            atp3 = attT_T(3)                         # PE
            attT_gather(3, atp3)                     # ACT
            recip(3)                                 # DVE
            out_chunk(2, 2)
            out_chunk(2, 3)
            out_chunk(3, 0)
            out_chunk(3, 1)
            out_chunk(3, 2)
            out_chunk(3, 3, split_tail=True)

    nc.compile()
    return nc


def _get_nc():
    if "nc" not in _CACHE:
        _CACHE["nc"] = _build_bass()
    return _CACHE["nc"]


def run(x, gamma, **run_kwargs):
    """Run on 8 cores; returns (results_list, BassKernelResults)."""
    from concourse.bass_utils import run_bass_kernel_spmd

    nc = _get_nc()
    x = np.ascontiguousarray(x, dtype=np.float32)
    gamma = np.ascontiguousarray(gamma, dtype=np.float32)
    in_maps = [
        {"x": np.ascontiguousarray(x[b].reshape(C, N)), "gamma": gamma}
        for b in range(B)
    ]
    res = run_bass_kernel_spmd(nc, in_maps, core_ids=list(range(B)), **run_kwargs)
    out = np.stack([r["out"] for r in res.results]).reshape(B, C, H, W)
    return out, res


def kernel(x, gamma):
    out, _ = run(x, gamma)
    return out.astype(np.float32)
